# revision 1
# baseline (speedup 1.0000x reference)
"""Multi-head attention (B=2, S=4096, D=512, H=8) on 8 TRN2 NeuronCores.

Sharding: data-parallel over (batch, query-chunk). Core i handles batch
i//4 and query rows (i%4)*1024 .. +1024 of that batch. Each core
computes Q projection for its query chunk, K/V projections for the full
batch (redundantly, 4 cores per batch), full attention for all 8 heads
over its queries, and the output projection for its rows. Output slices
are disjoint -> no collectives; host just concatenates.

Per-core device pipeline (transposed "d-major" layout, bf16 matmuls):
  1. Transposing DMAs (bf16 xbar mode) load x^T [i, t] directly; DMA
     queue ordered to minimize XBAR copy<->transpose mode flips.
  2. Q^T/K^T = W^T.T @ x^T; V = x^T.T @ Wv^T (natural layout), stored
     bf16 with a ones-column per head (V_aug). V and the K projection
     of later head pairs are interleaved into the attention loops.
  3. Per (head-pair, q-tile 512, k-chunk 128): scores^T [k,q] via 2
     row-packed matmuls (contraction d=64, heads at array rows 0-63 /
     64-127, concurrent in the PE array), one ACT exp [128,1024]
     psum->sbuf (scale=1/8), 2 attn@V matmuls lhsT=[V_h|1] [128,65] ->
     psum [65,512]; row 64 accumulates the softmax denominator.
     scores/exp for k+1 are emitted before attn@V of k (software
     pipeline) so the PE never waits on the exp.
  4. Normalize: copy po->sbuf (frees psum banks fast), reciprocal of
     row 64, then (deferred into the next block via pending_slow) a
     rank-1 broadcast matmul (f32r) and scalar_tensor_tensor multiply.
  5. Output projection: out[t,o] = sum_h aot_h^T.T @ Wo_h^T + bias.

Engines in steady state: ACT is saturated by the exp (the softmax
exponentials are the single largest floor: S*S*H*B/8 cores/128 lanes
/1.2GHz = 218us); PE array runs scores+attn@V at a similar rate.
"""

import numpy as np
import ml_dtypes

import concourse.bass as bass
import concourse.tile as tile
from concourse import bacc, mybir
from concourse.bass_utils import run_bass_kernel_spmd

F32 = mybir.dt.float32
F32R = mybir.dt.float32r
BF16 = mybir.dt.bfloat16
MUL = mybir.AluOpType.mult

B, S, D, H = 2, 4096, 512, 8
HD = D // H  # 64
NCORES = 8
QCH = B * S // NCORES  # 1024 query rows per core
TKV = S  # 4096 kv rows per core
IC = D // 128  # 4 contraction chunks
OC = D // 128  # 4 output chunks
QT = 512  # q tile (psum bank limit in fp32)
NQT = QCH // QT  # 2
KCH = TKV // 128  # 32 k chunks


def _build_program():
    nc = bacc.Bacc(
        "TRN2",
        target_bir_lowering=False,
        debug=False,
        enable_asserts=False,
        num_devices=NCORES,
    )
    xq = nc.dram_tensor("xq", [QCH, D], BF16, kind="ExternalInput").ap()
    xkv = nc.dram_tensor("xkv", [TKV, D], BF16, kind="ExternalInput").ap()
    wqt = nc.dram_tensor("wqt", [D, D], BF16, kind="ExternalInput").ap()
    wkt = nc.dram_tensor("wkt", [D, D], BF16, kind="ExternalInput").ap()
    wvt = nc.dram_tensor("wvt", [D, D], BF16, kind="ExternalInput").ap()
    wos = nc.dram_tensor("wos", [HD, H, D], BF16, kind="ExternalInput").ap()
    bqs = nc.dram_tensor("bqs", [128, OC], F32, kind="ExternalInput").ap()
    bks = nc.dram_tensor("bks", [128, OC], F32, kind="ExternalInput").ap()
    bvb = nc.dram_tensor("bvb", [128, D], F32, kind="ExternalInput").ap()
    bob = nc.dram_tensor("bob", [128, D], F32, kind="ExternalInput").ap()
    out = nc.dram_tensor("out", [QCH, D], F32, kind="ExternalOutput").ap()

    with tile.TileContext(nc) as tc:
        with (
            tc.tile_pool(name="consts", bufs=1) as consts,
            tc.tile_pool(name="persist", bufs=1) as persist,
            tc.tile_pool(name="pt", bufs=6) as pt_pool,
            tc.tile_pool(name="aot", bufs=1) as aot_pool,
            tc.tile_pool(name="osb", bufs=2) as osb_pool,
            tc.tile_pool(name="posb", bufs=4) as posb_pool,
            tc.tile_pool(name="small", bufs=4) as small_pool,
            # PSUM (8 banks): "sc" scores 2x2, "acc" 2x1 (transposes,
            # proj, pb, final), "po" 2x1 attn-out accumulators.
            tc.tile_pool(name="ps_sc", bufs=2, space="PSUM") as sc_pool,
            tc.tile_pool(name="ps_acc", bufs=2, space="PSUM") as acc_pool,
            tc.tile_pool(name="ps_po", bufs=2, space="PSUM") as po_pool,
        ):
            # ---- constants ----
            ones64f = consts.tile([1, HD], F32)
            nc.vector.memset(ones64f, 1.0)
            ones64 = consts.tile([1, HD], F32R)
            nc.vector.tensor_copy(ones64, ones64f)
            ones1b = consts.tile([1, 128], BF16)
            nc.vector.memset(ones1b, 1.0)


            # ---- persistent activations ----
            # x_kv^T split per DMA segment so transposing DMAs never
            # serialize against earlier segments' readers
            xtks = [
                persist.tile([128, IC, 1024], BF16, name=f"xtk{s}")
                for s in range(TKV // 1024)
            ]
            xtq = persist.tile([128, IC, QCH], BF16)  # x_q^T
            kt = persist.tile([128, OC, TKV], BF16)  # K^T [o-in-chunk, c, t]
            qt = persist.tile([128, OC, QCH], BF16)  # Q^T
            # V_aug: [t-in-chunk, t-chunk, head, 64 V cols + ones col]
            v_sb = persist.tile([128, KCH, H, HD + 1], BF16)
            nc.vector.memset(v_sb[:, :, :, HD : HD + 1], 1.0)

            # ---- phase T: transposing DMA loads x^T directly (bf16).
            # Order minimizes XBAR mode flips: xq+s0 transposes, then the
            # weight copies, then the remaining segments, then wo.
            SEG = 1024  # t-columns per transposed DMA segment
            wq_sb = consts.tile([128, IC, D], BF16)
            nc.sync.dma_start(wq_sb, wqt.rearrange("(c p) o -> p c o", p=128))
            wk_sb = consts.tile([128, IC, D], BF16)
            nc.sync.dma_start(wk_sb, wkt.rearrange("(c p) o -> p c o", p=128))
            wv_sb = consts.tile([128, IC, D], BF16)
            nc.sync.dma_start(wv_sb, wvt.rearrange("(c p) o -> p c o", p=128))
            bq_sb = consts.tile([128, OC], F32)
            nc.sync.dma_start(bq_sb, bqs)
            bk_sb = consts.tile([128, OC], F32)
            nc.sync.dma_start(bk_sb, bks)
            bvb_sb = consts.tile([128, D], F32)
            nc.sync.dma_start(bvb_sb, bvb)
            bv_row = consts.tile([1, D], BF16)
            nc.vector.tensor_copy(bv_row, bvb_sb[0:1, :])
            bob_sb = consts.tile([128, D], F32)
            nc.sync.dma_start(bob_sb, bob)
            for c in range(IC):
                nc.sync.dma_start_transpose(
                    xtq[:, c, :], xq[:, c * 128 : (c + 1) * 128]
                )
            for c in range(IC):
                nc.sync.dma_start_transpose(
                    xtks[0][:, c, :], xkv[0:SEG, c * 128 : (c + 1) * 128]
                )
            for s in range(1, TKV // SEG):
                for c in range(IC):
                    nc.sync.dma_start_transpose(
                        xtks[s][:, c, :],
                        xkv[s * SEG : (s + 1) * SEG, c * 128 : (c + 1) * 128],
                    )
            wo_sb = consts.tile([HD, H, D], BF16)
            nc.sync.dma_start(wo_sb, wos)

            def v_unit(j):
                # V rows for t-chunk j, all heads: [128 t, 512 d] + bias
                ps = acc_pool.tile([128, D], F32, tag="acc", name=f"v{j}")
                s, jj = divmod(j, 8)
                for i in range(IC):
                    nc.tensor.matmul(
                        ps,
                        xtks[s][:, i, jj * 128 : (jj + 1) * 128],
                        wv_sb[:, i, :],
                        start=(i == 0),
                        stop=(i == IC - 1),
                    )
                nc.vector.tensor_add(
                    v_sb[:, j, :, 0:HD],
                    ps.rearrange("p (h d) -> p h d", h=H),
                    bvb_sb.rearrange("p (h d) -> p h d", h=H),
                )

            def q_unit(c, tt):
                ps = acc_pool.tile([128, 512], F32, tag="acc", name=f"q{c}{tt}")
                for i in range(IC):
                    nc.tensor.matmul(
                        ps,
                        wq_sb[:, i, c * 128 : (c + 1) * 128],
                        xtq[:, i, tt * 512 : (tt + 1) * 512],
                        start=(i == 0),
                        stop=(i == IC - 1),
                    )
                nc.vector.tensor_scalar_add(
                    qt[:, c, tt * 512 : (tt + 1) * 512], ps, bq_sb[:, c : c + 1]
                )

            def k_unit(c, tt):
                ps = acc_pool.tile([128, 512], F32, tag="acc", name=f"k{c}_{tt}")
                s, ss = divmod(tt, 2)
                for i in range(IC):
                    nc.tensor.matmul(
                        ps,
                        wk_sb[:, i, c * 128 : (c + 1) * 128],
                        xtks[s][:, i, ss * 512 : (ss + 1) * 512],
                        start=(i == 0),
                        stop=(i == IC - 1),
                    )
                nc.vector.tensor_scalar_add(
                    kt[:, c, tt * 512 : (tt + 1) * 512], ps, bk_sb[:, c : c + 1]
                )

            def proj_units(c):
                for tt in range(TKV // 512):
                    yield lambda tt=tt: k_unit(c, tt)

            # projection units in x^T-segment dependency order; only Q
            # chunk 0 is needed before pair-0 attention starts
            for tt in range(QCH // 512):
                q_unit(0, tt)
            deferred_q = [
                (lambda c=c, tt=tt: q_unit(c, tt))
                for c in range(1, OC)
                for tt in range(QCH // 512)
            ]
            for tt in range(TKV // 512):
                k_unit(0, tt)
            for j in range(16):
                v_unit(j)

            aots = [aot_pool.tile([HD, H, QT], BF16, name=f"aot{qi}") for qi in (0, 1)]

            # ---- output projection for one 128-row block of q-tile qi ----
            def fin_unit(qi, t4):
                ps = acc_pool.tile([128, D], F32, tag="acc", name=f"f{qi}_{t4}")
                for h in range(H):
                    nc.tensor.matmul(
                        ps,
                        aots[qi][:, h, t4 * 128 : (t4 + 1) * 128],
                        wo_sb[:, h, :],
                        start=(h == 0),
                        stop=(h == H - 1),
                    )
                osb = osb_pool.tile([128, D], F32, tag="osb")
                nc.vector.tensor_add(osb, ps, bob_sb)
                t0 = qi * QT + t4 * 128
                nc.sync.dma_start(out[t0 : t0 + 128, :], osb)

            # ---- attention ----
            # Deferred work (next pair's K proj, normalize tails, output
            # projection) is drained one unit every other k-iteration so
            # the PE never stalls in-order on a slow dependency chain.
            pending = []
            pending_slow = []

            def norm2_unit(c, qi, hh, posb, recip):
                pb = acc_pool.tile(
                    [HD, QT], F32, tag="acc", name=f"pb{c}_{qi}_{hh}"
                )
                nc.tensor.matmul(pb, ones64, recip, start=True, stop=True)
                nc.vector.scalar_tensor_tensor(
                    aots[qi][:, 2 * c + hh, :],
                    pb,
                    1.0,
                    posb[0:HD, :],
                    op0=MUL,
                    op1=MUL,
                )

            for c in range(H // 2):
                if c < H // 2 - 1:
                    pending.extend(proj_units(c + 1))
                if c == 0:
                    pending.extend(deferred_q)
                for qi in range(NQT):
                    qs = qi * QT
                    po = [
                        po_pool.tile([HD + 1, QT], F32, tag="po", name=f"po{c}_{qi}_{hh}")
                        for hh in range(2)
                    ]

                    def scores_exp(k, qs=qs, c=c):
                        pss = sc_pool.tile([128, 2, QT], F32, tag="sc")
                        for hh in range(2):
                            off = hh * HD
                            nc.tensor.matmul(
                                pss[:, hh, :],
                                kt[off : off + HD, c, k * 128 : (k + 1) * 128],
                                qt[off : off + HD, c, qs : qs + QT],
                                start=True,
                                stop=True,
                            )
                        ptile = pt_pool.tile([128, 2, QT], BF16, tag="pt")
                        nc.scalar.activation(
                            ptile, pss, mybir.ActivationFunctionType.Exp,
                            scale=1.0 / np.sqrt(HD),
                        )
                        return ptile

                    # software pipeline: scores/exp for k+1 are emitted
                    # before attn@V of k so PE never stalls on the exp
                    ptile = scores_exp(0)
                    for k in range(KCH):
                        nxt = scores_exp(k + 1) if k + 1 < KCH else None
                        for hh in range(2):
                            nc.tensor.matmul(
                                po[hh],
                                v_sb[:, k, 2 * c + hh, :],
                                ptile[:, hh, :],
                                start=(k == 0),
                                stop=(k == KCH - 1),
                            )
                        ptile = nxt
                        if c == 0 and qi == 0 and k < 16:
                            v_unit(k + 16)
                        elif k >= 15 and k % 3 == 0 and pending_slow:
                            pending_slow.pop(0)()
                        elif k >= 3 and k % 2 == 1 and pending:
                            pending.pop(0)()
                    # free the po banks quickly; defer the slow
                    # recip/broadcast/multiply chain into the next block
                    posbs = []
                    for hh in range(2):
                        posb = posb_pool.tile(
                            [HD + 1, QT], F32, tag="posb", name=f"posb{c}_{qi}_{hh}"
                        )
                        nc.vector.tensor_copy(posb, po[hh])
                        posbs.append(posb)
                    for hh in range(2):
                        recip = small_pool.tile([1, QT], F32R, tag="recip")
                        with nc.allow_low_precision(reason="f32r recip"):
                            nc.vector.reciprocal(recip, posbs[hh][HD : HD + 1, :])
                        pending_slow.append(
                            lambda c=c, qi=qi, hh=hh, posb=posbs[hh], recip=recip: (
                                norm2_unit(c, qi, hh, posb, recip)
                            )
                        )
                    if c == H // 2 - 1 and qi == 0:
                        pending_slow.extend(
                            lambda t4=t4: fin_unit(0, t4) for t4 in range(QT // 128)
                        )
            for u in pending + pending_slow:
                u()
            for t4 in range(QT // 128):
                fin_unit(1, t4)

    nc.compile()
    return nc


_NC_CACHE = None


def _get_program():
    global _NC_CACHE
    if _NC_CACHE is None:
        _NC_CACHE = _build_program()
    return _NC_CACHE


def prepare_in_maps(x, Wq, bq, Wk, bk, Wv, bv, Wo, bo):
    bf = ml_dtypes.bfloat16
    x = np.ascontiguousarray(np.asarray(x, dtype=np.float32)).astype(bf)
    sh = {
        "wqt": np.ascontiguousarray(np.asarray(Wq, np.float32).T).astype(bf),
        "wkt": np.ascontiguousarray(np.asarray(Wk, np.float32).T).astype(bf),
        "wvt": np.ascontiguousarray(np.asarray(Wv, np.float32).T).astype(bf),
        "wos": np.ascontiguousarray(
            np.asarray(Wo, np.float32).T.reshape(H, HD, D).transpose(1, 0, 2)
        ).astype(bf),
        "bqs": np.ascontiguousarray(np.asarray(bq, np.float32).reshape(OC, 128).T),
        "bks": np.ascontiguousarray(np.asarray(bk, np.float32).reshape(OC, 128).T),
        "bvb": np.ascontiguousarray(
            np.broadcast_to(np.asarray(bv, np.float32), (128, D))
        ),
        "bob": np.ascontiguousarray(
            np.broadcast_to(np.asarray(bo, np.float32), (128, D))
        ),
    }
    in_maps = []
    for core in range(NCORES):
        b = core // (NCORES // B)
        qs = (core % (NCORES // B)) * QCH
        m = dict(sh)
        m["xq"] = np.ascontiguousarray(x[b, qs : qs + QCH, :])
        m["xkv"] = np.ascontiguousarray(x[b])
        in_maps.append(m)
    return in_maps


def assemble(results):
    out = np.empty((B, S, D), dtype=np.float32)
    for core in range(NCORES):
        b = core // (NCORES // B)
        qs = (core % (NCORES // B)) * QCH
        out[b, qs : qs + QCH, :] = results[core]["out"]
    return out


def kernel(x, Wq, bq, Wk, bk, Wv, bv, Wo, bo):
    in_maps = prepare_in_maps(x, Wq, bq, Wk, bk, Wv, bv, Wo, bo)
    nc = _get_program()
    res = run_bass_kernel_spmd(nc, in_maps, core_ids=list(range(NCORES)))
    return assemble(res.results)



# revision 21
# speedup vs baseline: 1.1127x; 1.1127x over previous
"""Multi-head attention (B=2, S=4096, D=512, H=8) on 8 TRN2 NeuronCores.

Sharding: data-parallel over (batch, query-chunk). Core i handles batch
i//4 and query rows (i%4)*1024 .. +1024 of that batch. Each core
computes Q projection for its query chunk, K/V projections for the full
batch (redundantly, 4 cores per batch), full attention for all 8 heads
over its queries, and the output projection for its rows. Output slices
are disjoint -> no collectives; host just concatenates.

Per-core device pipeline (transposed "d-major" layout, bf16 matmuls):
  1. Transposing DMAs (bf16 xbar mode) load x^T on the sync queue while
     weights/biases stream in parallel on the scalar-engine HWDGE queue.
  2. Q^T/K^T = W^T.T @ x^T; V = x^T.T @ Wv^T (natural layout), stored
     bf16 with a ones-column per head (V_aug).
  3. Per (head-pair, q-tile 512, k-chunk 128): scores^T [k,q] via 2
     row-packed matmuls, one ACT exp [128,1024] psum->sbuf (scale=1/8),
     2 attn@V matmuls lhsT=[V_h|1] [128,65] -> psum [65,512]; row 64
     accumulates the softmax denominator. scores/exp for k+1 are
     emitted before attn@V of k (software pipeline).
  4. All deferred work (projections for later head-pairs, softmax
     normalization, output projection) is queued as ~1-matmul closures
     and drained one budget unit per k-iteration so the PE load stays
     smooth and the ACT engine (the exp is the per-core floor:
     256 x 1.34us) never starves.
  5. Normalize per (pair, q-tile): fast approx reciprocal of the
     denominator row, rank-1 broadcast matmul, scalar_tensor_tensor.
  6. Output projection runs per head-pair into a PSUM tile and is
     accumulated into a persistent SBUF buffer by the vector engine, so
     the final tail after the last exp is only a few microseconds.

Engines in steady state: ACT saturated by the exp; PE runs scores,
attn@V and the dripped projection matmuls just under the ACT rate.
"""

import numpy as np
import ml_dtypes

import concourse.bass as bass
import concourse.tile as tile
from concourse import bacc, mybir
from concourse.bass_utils import run_bass_kernel_spmd

F32 = mybir.dt.float32
F32R = mybir.dt.float32r
BF16 = mybir.dt.bfloat16
MUL = mybir.AluOpType.mult

B, S, D, H = 2, 4096, 512, 8
HD = D // H  # 64
NCORES = 8
QCH = B * S // NCORES  # 1024 query rows per core
TKV = S  # 4096 kv rows per core
IC = D // 128  # 4 contraction chunks
OC = D // 128  # 4 output chunks
QT = 512  # q tile (psum bank limit in fp32)
NQT = QCH // QT  # 2
KCH = TKV // 128  # 32 k chunks


def _build_program():
    nc = bacc.Bacc(
        "TRN2",
        target_bir_lowering=False,
        debug=False,
        enable_asserts=False,
        num_devices=NCORES,
    )
    xq = nc.dram_tensor("xq", [QCH, D], BF16, kind="ExternalInput").ap()
    xkv = nc.dram_tensor("xkv", [TKV, D], BF16, kind="ExternalInput").ap()
    wqt = nc.dram_tensor("wqt", [D, D], BF16, kind="ExternalInput").ap()
    wkt = nc.dram_tensor("wkt", [D, D], BF16, kind="ExternalInput").ap()
    wvt = nc.dram_tensor("wvt", [D, D], BF16, kind="ExternalInput").ap()
    wos = nc.dram_tensor("wos", [HD, H, D], BF16, kind="ExternalInput").ap()
    bqs = nc.dram_tensor("bqs", [128, OC], F32, kind="ExternalInput").ap()
    bks = nc.dram_tensor("bks", [128, OC], F32, kind="ExternalInput").ap()
    bvb = nc.dram_tensor("bvb", [128, D], F32, kind="ExternalInput").ap()
    bob = nc.dram_tensor("bob", [128, D], F32, kind="ExternalInput").ap()
    out = nc.dram_tensor("out", [QCH, D], F32, kind="ExternalOutput").ap()

    with tile.TileContext(nc) as tc:
        with (
            tc.tile_pool(name="consts", bufs=1) as consts,
            tc.tile_pool(name="persist", bufs=1) as persist,
            tc.tile_pool(name="pt", bufs=4) as pt_pool,
            tc.tile_pool(name="aot", bufs=2) as aot_pool,
            tc.tile_pool(name="posb", bufs=4) as posb_pool,
            # PSUM (8 banks): "sc" scores 2x2, "acc" 2x1 (proj, pb,
            # fin), "po" 2x1 attn-out accumulators.
            tc.tile_pool(name="ps_sc", bufs=2, space="PSUM") as sc_pool,
            tc.tile_pool(name="ps_acc", bufs=2, space="PSUM") as acc_pool,
            tc.tile_pool(name="ps_po", bufs=2, space="PSUM") as po_pool,
        ):
            # ---- constants ----
            ones64 = consts.tile([1, HD], BF16)
            nc.vector.memset(ones64, 1.0)
            # 2 rotating reciprocal slots on partition 0 (matmul rhs needs
            # base partition 0); region deps serialize reuse correctly.
            # f32 approx-recip result, rounded to bf16 for the rank-1
            # broadcast matmul (a ~2^-9 per-column scale, well in budget).
            recips = consts.tile([1, 2, QT], F32, name="recips")
            recipb = consts.tile([1, 2, QT], BF16, name="recipb")
            recip_slot = [0]

            # ---- persistent activations ----
            xtks = [
                persist.tile([128, IC, 1024], BF16, name=f"xtk{s}")
                for s in range(TKV // 1024)
            ]
            xtq = persist.tile([128, IC, QCH], BF16)  # x_q^T
            kt = persist.tile([128, OC, TKV], BF16)  # K^T [o-in-chunk, c, t]
            qt = persist.tile([128, OC, QCH], BF16)  # Q^T
            # V_aug: [t-in-chunk, t-chunk, head, 64 V cols + ones col]
            v_sb = persist.tile([128, KCH, H, HD + 1], BF16)
            nc.vector.memset(v_sb[:, :, :, HD : HD + 1], 1.0)
            # output accumulator [q-in-chunk, qi, t4, D] f32
            out_acc = persist.tile([128, NQT, QT // 128, D], F32)

            # ---- DMA (single sync HWDGE queue), ordered so the prefix
            # projections can start as early as possible: xq transposes,
            # Q weights, x_kv segment 0 transposes, K/V weights, the
            # remaining segments, then the output-projection weights.
            for c in range(IC):
                nc.sync.dma_start_transpose(
                    xtq[:, c, :], xq[:, c * 128 : (c + 1) * 128]
                )
            wq_sb = consts.tile([128, IC, D], BF16)
            nc.sync.dma_start(wq_sb, wqt.rearrange("(c p) o -> p c o", p=128))
            bq_sb = consts.tile([128, OC], F32)
            nc.sync.dma_start(bq_sb, bqs)
            SEG = 1024
            for c in range(IC):
                nc.sync.dma_start_transpose(
                    xtks[0][:, c, :], xkv[0:SEG, c * 128 : (c + 1) * 128]
                )
            wk_sb = consts.tile([128, IC, D], BF16)
            nc.sync.dma_start(wk_sb, wkt.rearrange("(c p) o -> p c o", p=128))
            bk_sb = consts.tile([128, OC], F32)
            nc.sync.dma_start(bk_sb, bks)
            wv_sb = consts.tile([128, IC, D], BF16)
            nc.sync.dma_start(wv_sb, wvt.rearrange("(c p) o -> p c o", p=128))
            bvb_sb = consts.tile([128, D], F32)
            nc.sync.dma_start(bvb_sb, bvb)
            bob_sb = consts.tile([128, D], F32)
            nc.sync.dma_start(bob_sb, bob)
            for s in range(1, TKV // SEG):
                for c in range(IC):
                    nc.sync.dma_start_transpose(
                        xtks[s][:, c, :],
                        xkv[s * SEG : (s + 1) * SEG, c * 128 : (c + 1) * 128],
                    )
            wo_sb = consts.tile([HD, H, D], BF16)
            nc.sync.dma_start(wo_sb, wos)

            # ---- projection units ----
            def k_unit(c, tt):
                # atomic 512-col K^T unit (used in the first block, where
                # closure interleaving with v_unit allocs must stay
                # one-tile-at-a-time)
                ps = acc_pool.tile([128, 512], F32, tag="acc", name=f"k{c}_{tt}")
                s, ss = divmod(tt, 2)
                for i in range(IC):
                    nc.tensor.matmul(
                        ps,
                        wk_sb[:, i, c * 128 : (c + 1) * 128],
                        xtks[s][:, i, ss * 512 : (ss + 1) * 512],
                        start=(i == 0),
                        stop=(i == IC - 1),
                    )
                nc.vector.tensor_scalar_add(
                    kt[:, c, tt * 512 : (tt + 1) * 512], ps, bk_sb[:, c : c + 1]
                )

            def v_unit(j):
                # V rows for t-chunk j, all heads: [128 t, 512 d] + bias
                ps = acc_pool.tile([128, D], F32, tag="acc", name=f"v{j}")
                s, jj = divmod(j, 8)
                for i in range(IC):
                    nc.tensor.matmul(
                        ps,
                        xtks[s][:, i, jj * 128 : (jj + 1) * 128],
                        wv_sb[:, i, :],
                        start=(i == 0),
                        stop=(i == IC - 1),
                    )
                nc.vector.tensor_add(
                    v_sb[:, j, :, 0:HD],
                    ps.rearrange("p (h d) -> p h d", h=H),
                    bvb_sb.rearrange("p (h d) -> p h d", h=H),
                )

            def q_pair_closures(c):
                # q chunks for both 512-col tiles, weights shared per i
                st = {}

                def mk(i):
                    def go():
                        if i == 0:
                            st["a"] = acc_pool.tile(
                                [128, 512], F32, tag="acc", name=f"qa{c}"
                            )
                            st["b"] = acc_pool.tile(
                                [128, 512], F32, tag="acc", name=f"qb{c}"
                            )
                        w = wq_sb[:, i, c * 128 : (c + 1) * 128]
                        nc.tensor.matmul(
                            st["a"], w, xtq[:, i, 0:512],
                            start=(i == 0), stop=(i == IC - 1),
                        )
                        nc.tensor.matmul(
                            st["b"], w, xtq[:, i, 512:1024],
                            start=(i == 0), stop=(i == IC - 1),
                        )
                        if i == IC - 1:
                            nc.vector.tensor_scalar_add(
                                qt[:, c, 0:512], st["a"], bq_sb[:, c : c + 1]
                            )
                            nc.vector.tensor_scalar_add(
                                qt[:, c, 512:1024], st["b"], bq_sb[:, c : c + 1]
                            )
                    return (2, go)

                return [mk(i) for i in range(IC)]

            def k_pair_closures(c, p):
                # K^T chunk c, t-cols [p*1024, (p+1)*1024), weights shared
                st = {}

                def mk(i):
                    def go():
                        if i == 0:
                            st["a"] = acc_pool.tile(
                                [128, 512], F32, tag="acc", name=f"ka{c}_{p}"
                            )
                            st["b"] = acc_pool.tile(
                                [128, 512], F32, tag="acc", name=f"kb{c}_{p}"
                            )
                        w = wk_sb[:, i, c * 128 : (c + 1) * 128]
                        nc.tensor.matmul(
                            st["a"], w, xtks[p][:, i, 0:512],
                            start=(i == 0), stop=(i == IC - 1),
                        )
                        nc.tensor.matmul(
                            st["b"], w, xtks[p][:, i, 512:1024],
                            start=(i == 0), stop=(i == IC - 1),
                        )
                        if i == IC - 1:
                            t0 = p * 1024
                            nc.vector.tensor_scalar_add(
                                kt[:, c, t0 : t0 + 512], st["a"],
                                bk_sb[:, c : c + 1],
                            )
                            nc.vector.tensor_scalar_add(
                                kt[:, c, t0 + 512 : t0 + 1024], st["b"],
                                bk_sb[:, c : c + 1],
                            )
                    return (2, go)

                return [mk(i) for i in range(IC)]

            # ---- normalize + output projection closures for one block ----
            def normfin_closures(c, qi):
                st = {}

                def norm_a(hh, po):
                    # inline at block end: DVE-only, frees the po psum
                    # banks immediately (no PE cost)
                    posb = posb_pool.tile(
                        [HD + 1, QT], F32, tag="posb", name=f"po{c}_{qi}_{hh}"
                    )
                    nc.vector.tensor_copy(posb, po)
                    sl = recip_slot[0]
                    recip_slot[0] = (sl + 1) % 2
                    with nc.allow_low_precision(reason="recip of softmax denom"):
                        nc.vector.reciprocal(
                            recips[0:1, sl, :], posb[HD : HD + 1, :]
                        )
                    nc.vector.tensor_copy(recipb[0:1, sl, :], recips[0:1, sl, :])
                    st[("posb", hh)] = posb
                    st[("recip", hh)] = recipb[0:1, sl, :]

                def mk_norm_b(hh):
                    def go():
                        if hh == 0:
                            st["aot"] = aot_pool.tile(
                                [HD, 2, QT], BF16, name=f"aot{c}_{qi}"
                            )
                        pb = acc_pool.tile(
                            [HD, QT], F32, tag="acc", name=f"pb{c}_{qi}_{hh}"
                        )
                        nc.tensor.matmul(
                            pb, ones64, st[("recip", hh)],
                            start=True, stop=True,
                        )
                        nc.vector.scalar_tensor_tensor(
                            st["aot"][:, hh, :],
                            pb,
                            1.0,
                            st[("posb", hh)][0:HD, :],
                            op0=MUL,
                            op1=MUL,
                        )
                    return (1, go)

                def mk_fin(t4):
                    def go():
                        ps = acc_pool.tile(
                            [128, D], F32, tag="acc", name=f"f{c}_{qi}_{t4}"
                        )
                        for hh in range(2):
                            nc.tensor.matmul(
                                ps,
                                st["aot"][:, hh, t4 * 128 : (t4 + 1) * 128],
                                wo_sb[:, 2 * c + hh, :],
                                start=(hh == 0),
                                stop=(hh == 1),
                            )
                        dst = out_acc[:, qi, t4, :]
                        if c == 0:
                            nc.vector.tensor_add(dst, ps, bob_sb)
                        else:
                            nc.vector.tensor_add(dst, dst, ps)
                        if c == H // 2 - 1:
                            t0 = qi * QT + t4 * 128
                            nc.sync.dma_start(out[t0 : t0 + 128, :], dst)
                    return (2, go)

                return norm_a, mk_norm_b, mk_fin

            # ---- pending-work queue: (mm_cost, closure), drained with a
            # per-iteration budget so PE load stays smooth.
            pending = []
            budget = [0.0]

            def drain(rate):
                budget[0] = min(budget[0] + rate, 4.0)
                while pending and pending[0][0] <= budget[0]:
                    cost, fn = pending.pop(0)
                    fn()
                    budget[0] -= cost

            # ---- prefix: minimum projections before attention (only
            # x^T segment 0 required, so PE starts as soon as the first
            # transposes land)
            for cl in q_pair_closures(0):
                cl[1]()
            for cl in k_pair_closures(0, 0):
                cl[1]()
            for j in range(8):
                v_unit(j)
            # remaining K chunk-0 units drain inside the first block
            pending.extend(
                (4, (lambda tt=tt: k_unit(0, tt))) for tt in range(2, 8)
            )

            # ---- attention ----
            for c in range(H // 2):
                for qi in range(NQT):
                    if qi == 1 and c + 1 < H // 2:
                        pending.extend(q_pair_closures(c + 1))
                        for p in range(4):
                            pending.extend(k_pair_closures(c + 1, p))
                    qs = qi * QT
                    po = [
                        po_pool.tile(
                            [HD + 1, QT], F32, tag="po", name=f"po{c}_{qi}_{hh}"
                        )
                        for hh in range(2)
                    ]

                    def scores_exp(k, qs=qs, c=c):
                        pss = sc_pool.tile([128, 2, QT], F32, tag="sc")
                        for hh in range(2):
                            off = hh * HD
                            nc.tensor.matmul(
                                pss[:, hh, :],
                                kt[off : off + HD, c, k * 128 : (k + 1) * 128],
                                qt[off : off + HD, c, qs : qs + QT],
                                start=True,
                                stop=True,
                            )
                        ptile = pt_pool.tile([128, 2, QT], BF16, tag="pt")
                        nc.scalar.activation(
                            ptile, pss, mybir.ActivationFunctionType.Exp,
                            scale=1.0 / np.sqrt(HD),
                        )
                        return ptile

                    first = c == 0 and qi == 0
                    ptile = scores_exp(0)
                    for k in range(KCH):
                        nxt = scores_exp(k + 1) if k + 1 < KCH else None
                        for hh in range(2):
                            nc.tensor.matmul(
                                po[hh],
                                v_sb[:, k, 2 * c + hh, :],
                                ptile[:, hh, :],
                                start=(k == 0),
                                stop=(k == KCH - 1),
                            )
                        ptile = nxt
                        if first and k < 24:
                            v_unit(k + 8)
                        drain(1.0 if k < 28 else 2.0)
                    # queue normalization + output projection for this block
                    norm_a, mk_b, mk_f = normfin_closures(c, qi)
                    norm_a(0, po[0])
                    norm_a(1, po[1])
                    pending.append(mk_b(0))
                    pending.append(mk_b(1))
                    for t4 in range(QT // 128):
                        pending.append(mk_f(t4))
            while pending:
                pending.pop(0)[1]()

    nc.compile()
    return nc


_NC_CACHE = None


def _get_program():
    global _NC_CACHE
    if _NC_CACHE is None:
        _NC_CACHE = _build_program()
    return _NC_CACHE


def prepare_in_maps(x, Wq, bq, Wk, bk, Wv, bv, Wo, bo):
    bf = ml_dtypes.bfloat16
    x = np.ascontiguousarray(np.asarray(x, dtype=np.float32)).astype(bf)
    sh = {
        "wqt": np.ascontiguousarray(np.asarray(Wq, np.float32).T).astype(bf),
        "wkt": np.ascontiguousarray(np.asarray(Wk, np.float32).T).astype(bf),
        "wvt": np.ascontiguousarray(np.asarray(Wv, np.float32).T).astype(bf),
        "wos": np.ascontiguousarray(
            np.asarray(Wo, np.float32).T.reshape(H, HD, D).transpose(1, 0, 2)
        ).astype(bf),
        "bqs": np.ascontiguousarray(np.asarray(bq, np.float32).reshape(OC, 128).T),
        "bks": np.ascontiguousarray(np.asarray(bk, np.float32).reshape(OC, 128).T),
        "bvb": np.ascontiguousarray(
            np.broadcast_to(np.asarray(bv, np.float32), (128, D))
        ),
        "bob": np.ascontiguousarray(
            np.broadcast_to(np.asarray(bo, np.float32), (128, D))
        ),
    }
    in_maps = []
    for core in range(NCORES):
        b = core // (NCORES // B)
        qs = (core % (NCORES // B)) * QCH
        m = dict(sh)
        m["xq"] = np.ascontiguousarray(x[b, qs : qs + QCH, :])
        m["xkv"] = np.ascontiguousarray(x[b])
        in_maps.append(m)
    return in_maps


def assemble(results):
    out = np.empty((B, S, D), dtype=np.float32)
    for core in range(NCORES):
        b = core // (NCORES // B)
        qs = (core % (NCORES // B)) * QCH
        out[b, qs : qs + QCH, :] = results[core]["out"]
    return out


def kernel(x, Wq, bq, Wk, bk, Wv, bv, Wo, bo):
    in_maps = prepare_in_maps(x, Wq, bq, Wk, bk, Wv, bv, Wo, bo)
    nc = _get_program()
    res = run_bass_kernel_spmd(nc, in_maps, core_ids=list(range(NCORES)))
    return assemble(res.results)


# revision 31
# speedup vs baseline: 1.3013x; 1.1695x over previous
"""Multi-head attention (B=2, S=4096, D=512, H=8) on 8 TRN2 NeuronCores.

Sharding: data-parallel over (batch, query-chunk). Core i handles batch
i//4 and query rows (i%4)*1024 .. +1024 of that batch. Each core
computes Q projection for its query chunk, K/V projections for the full
batch (redundantly, 4 cores per batch), full attention for all 8 heads
over its queries, and the output projection for its rows. Output slices
are disjoint -> no collectives; host just concatenates.

Per-core device pipeline (transposed "d-major" layout, bf16 matmuls):
  1. Transposing DMAs (bf16 xbar mode) load x^T on the sync queue while
     weights/biases stream in parallel on the scalar-engine HWDGE queue.
  2. Q^T/K^T = W^T.T @ x^T; V = x^T.T @ Wv^T (natural layout), stored
     bf16 with a ones-column per head (V_aug).
  3. Per (head-pair, q-tile 512, k-chunk 128): scores^T [k,q] via 2
     row-packed matmuls, one ACT exp [128,1024] psum->sbuf (scale=1/8),
     2 attn@V matmuls lhsT=[V_h|1] [128,65] -> psum [65,512]; row 64
     accumulates the softmax denominator. scores/exp for k+1 are
     emitted before attn@V of k (software pipeline).
  4. All deferred work (projections for later head-pairs, softmax
     normalization, output projection) is queued as ~1-matmul closures
     and drained one budget unit per k-iteration so the PE load stays
     smooth and the ACT engine (the exp is the per-core floor:
     256 x 1.34us) never starves.
  5. Normalize per (pair, q-tile): fast approx reciprocal of the
     denominator row, rank-1 broadcast matmul, scalar_tensor_tensor.
  6. Output projection runs per head-pair into a PSUM tile and is
     accumulated into a persistent SBUF buffer by the vector engine, so
     the final tail after the last exp is only a few microseconds.

Engines in steady state: ACT saturated by the exp; PE runs scores,
attn@V and the dripped projection matmuls just under the ACT rate.
"""

import numpy as np
import ml_dtypes

import concourse.bass as bass
import concourse.tile as tile
from concourse import bacc, mybir
from concourse.bass_utils import run_bass_kernel_spmd

F32 = mybir.dt.float32
F32R = mybir.dt.float32r
BF16 = mybir.dt.bfloat16
MUL = mybir.AluOpType.mult
ADD = mybir.AluOpType.add

B, S, D, H = 2, 4096, 512, 8
HD = D // H  # 64
NCORES = 8
QCH = B * S // NCORES  # 1024 query rows per core
TKV = S  # 4096 kv rows per core
IC = D // 128  # 4 contraction chunks
OC = D // 128  # 4 output chunks
QT = 512  # q tile (psum bank limit in fp32)
NQT = QCH // QT  # 2
KCH = TKV // 128  # 32 k chunks


def _build_program():
    nc = bacc.Bacc(
        "TRN2",
        target_bir_lowering=False,
        debug=False,
        enable_asserts=False,
        num_devices=NCORES,
    )
    xq = nc.dram_tensor("xq", [QCH, D], BF16, kind="ExternalInput").ap()
    xkv = nc.dram_tensor("xkv", [TKV, D], BF16, kind="ExternalInput").ap()
    wqt = nc.dram_tensor("wqt", [D, D], BF16, kind="ExternalInput").ap()
    wkt = nc.dram_tensor("wkt", [D, D], BF16, kind="ExternalInput").ap()
    wvt = nc.dram_tensor("wvt", [D, D], BF16, kind="ExternalInput").ap()
    wos = nc.dram_tensor("wos", [HD, H, D], BF16, kind="ExternalInput").ap()
    bqs = nc.dram_tensor("bqs", [128, OC], F32, kind="ExternalInput").ap()
    bks = nc.dram_tensor("bks", [128, OC], F32, kind="ExternalInput").ap()
    bvb = nc.dram_tensor("bvb", [128, D], F32, kind="ExternalInput").ap()
    bob = nc.dram_tensor("bob", [128, D], F32, kind="ExternalInput").ap()
    out = nc.dram_tensor("out", [QCH, D], F32, kind="ExternalOutput").ap()
    # DRAM bounce buffer for transposing softmax-denominator rows
    dsc = nc.dram_tensor("denscratch", [2, 2, QT], F32).ap()

    with tile.TileContext(nc) as tc:
        with (
            tc.tile_pool(name="consts", bufs=1) as consts,
            tc.tile_pool(name="persist", bufs=1) as persist,
            tc.tile_pool(name="pt", bufs=4) as pt_pool,
            tc.tile_pool(name="aot", bufs=2) as aot_pool,
            # PSUM (8 banks): "sc" scores 2x2, "acc" 2x1 (proj, pb,
            # fin), "po" 2x1 attn-out accumulators.
            tc.tile_pool(name="ps_sc", bufs=2, space="PSUM") as sc_pool,
            tc.tile_pool(name="ps_acc", bufs=2, space="PSUM") as acc_pool,
            tc.tile_pool(name="ps_po", bufs=2, space="PSUM") as po_pool,
        ):
            # ---- constants ----
            # Denominator path: po row HD (the ones-column sum) is DMA'd
            # PSUM->DRAM->SBUF so it lands partition-major [128, t4]; the
            # reciprocal then runs 128-wide and normalization folds into
            # the per-head output-projection accumulate as a per-partition
            # scalar multiply. 2 rotating slots x 2 heads.
            dent = consts.tile([128, 2, 2, QT // 128], F32, name="dent")
            dentr = consts.tile([128, 2, 2, QT // 128], F32, name="dentr")
            den_sb = consts.tile([1, 2, QT], F32, name="den_sb")
            den_slot = [0]

            # ---- persistent activations ----
            xtks = [
                persist.tile([128, IC, 1024], BF16, name=f"xtk{s}")
                for s in range(TKV // 1024)
            ]
            xtq = persist.tile([128, IC, QCH], BF16)  # x_q^T
            kt = persist.tile([128, OC, TKV], BF16)  # K^T [o-in-chunk, c, t]
            qt = persist.tile([128, OC, QCH], BF16)  # Q^T
            # V_aug: [t-in-chunk, t-chunk, head, 64 V cols + ones col]
            v_sb = persist.tile([128, KCH, H, HD + 1], BF16)
            nc.vector.memset(v_sb[:, :, :, HD : HD + 1], 1.0)
            # output accumulator [q-in-chunk, qi, t4, D] f32
            out_acc = persist.tile([128, NQT, QT // 128, D], F32)

            # ---- DMA (single sync HWDGE queue), ordered so the prefix
            # projections can start as early as possible: xq transposes,
            # Q weights, x_kv segment 0 transposes, K/V weights, the
            # remaining segments, then the output-projection weights.
            for c in range(IC):
                nc.sync.dma_start_transpose(
                    xtq[:, c, :], xq[:, c * 128 : (c + 1) * 128]
                )
            wq_sb = consts.tile([128, IC, D], BF16)
            nc.sync.dma_start(wq_sb, wqt.rearrange("(c p) o -> p c o", p=128))
            bq_sb = consts.tile([128, OC], F32)
            nc.sync.dma_start(bq_sb, bqs)
            SEG = 1024
            for c in range(IC):
                nc.sync.dma_start_transpose(
                    xtks[0][:, c, :], xkv[0:SEG, c * 128 : (c + 1) * 128]
                )
            wk_sb = consts.tile([128, IC, D], BF16)
            nc.sync.dma_start(wk_sb, wkt.rearrange("(c p) o -> p c o", p=128))
            bk_sb = consts.tile([128, OC], F32)
            nc.sync.dma_start(bk_sb, bks)
            wv_sb = consts.tile([128, IC, D], BF16)
            nc.sync.dma_start(wv_sb, wvt.rearrange("(c p) o -> p c o", p=128))
            bvb_sb = consts.tile([128, D], F32)
            nc.sync.dma_start(bvb_sb, bvb)
            bob_sb = consts.tile([128, D], F32)
            nc.sync.dma_start(bob_sb, bob)
            for s in range(1, TKV // SEG):
                for c in range(IC):
                    nc.sync.dma_start_transpose(
                        xtks[s][:, c, :],
                        xkv[s * SEG : (s + 1) * SEG, c * 128 : (c + 1) * 128],
                    )
            wo_sb = consts.tile([HD, H, D], BF16)
            nc.sync.dma_start(wo_sb, wos)

            # ---- projection units ----
            def k_unit(c, tt):
                # atomic 512-col K^T unit (used in the first block, where
                # closure interleaving with v_unit allocs must stay
                # one-tile-at-a-time)
                ps = acc_pool.tile([128, 512], F32, tag="acc", name=f"k{c}_{tt}")
                s, ss = divmod(tt, 2)
                for i in range(IC):
                    nc.tensor.matmul(
                        ps,
                        wk_sb[:, i, c * 128 : (c + 1) * 128],
                        xtks[s][:, i, ss * 512 : (ss + 1) * 512],
                        start=(i == 0),
                        stop=(i == IC - 1),
                    )
                nc.vector.tensor_scalar_add(
                    kt[:, c, tt * 512 : (tt + 1) * 512], ps, bk_sb[:, c : c + 1]
                )

            def v_unit(j):
                # V rows for t-chunk j, all heads: [128 t, 512 d] + bias
                ps = acc_pool.tile([128, D], F32, tag="acc", name=f"v{j}")
                s, jj = divmod(j, 8)
                for i in range(IC):
                    nc.tensor.matmul(
                        ps,
                        xtks[s][:, i, jj * 128 : (jj + 1) * 128],
                        wv_sb[:, i, :],
                        start=(i == 0),
                        stop=(i == IC - 1),
                    )
                nc.vector.tensor_add(
                    v_sb[:, j, :, 0:HD],
                    ps.rearrange("p (h d) -> p h d", h=H),
                    bvb_sb.rearrange("p (h d) -> p h d", h=H),
                )

            def q_pair_closures(c):
                # q chunks for both 512-col tiles, weights shared per i
                st = {}

                def mk(i):
                    def go():
                        if i == 0:
                            st["a"] = acc_pool.tile(
                                [128, 512], F32, tag="acc", name=f"qa{c}"
                            )
                            st["b"] = acc_pool.tile(
                                [128, 512], F32, tag="acc", name=f"qb{c}"
                            )
                        w = wq_sb[:, i, c * 128 : (c + 1) * 128]
                        nc.tensor.matmul(
                            st["a"], w, xtq[:, i, 0:512],
                            start=(i == 0), stop=(i == IC - 1),
                        )
                        nc.tensor.matmul(
                            st["b"], w, xtq[:, i, 512:1024],
                            start=(i == 0), stop=(i == IC - 1),
                        )
                        if i == IC - 1:
                            nc.vector.tensor_scalar_add(
                                qt[:, c, 0:512], st["a"], bq_sb[:, c : c + 1]
                            )
                            nc.vector.tensor_scalar_add(
                                qt[:, c, 512:1024], st["b"], bq_sb[:, c : c + 1]
                            )
                    return (2, go)

                return [mk(i) for i in range(IC)]

            def k_pair_closures(c, p):
                # K^T chunk c, t-cols [p*1024, (p+1)*1024), weights shared
                st = {}

                def mk(i):
                    def go():
                        if i == 0:
                            st["a"] = acc_pool.tile(
                                [128, 512], F32, tag="acc", name=f"ka{c}_{p}"
                            )
                            st["b"] = acc_pool.tile(
                                [128, 512], F32, tag="acc", name=f"kb{c}_{p}"
                            )
                        w = wk_sb[:, i, c * 128 : (c + 1) * 128]
                        nc.tensor.matmul(
                            st["a"], w, xtks[p][:, i, 0:512],
                            start=(i == 0), stop=(i == IC - 1),
                        )
                        nc.tensor.matmul(
                            st["b"], w, xtks[p][:, i, 512:1024],
                            start=(i == 0), stop=(i == IC - 1),
                        )
                        if i == IC - 1:
                            t0 = p * 1024
                            nc.vector.tensor_scalar_add(
                                kt[:, c, t0 : t0 + 512], st["a"],
                                bk_sb[:, c : c + 1],
                            )
                            nc.vector.tensor_scalar_add(
                                kt[:, c, t0 + 512 : t0 + 1024], st["b"],
                                bk_sb[:, c : c + 1],
                            )
                    return (2, go)

                return [mk(i) for i in range(IC)]

            # ---- normalize + output projection closures for one block ----
            def normfin_closures(c, qi):
                st = {}
                sl = den_slot[0]
                den_slot[0] = (sl + 1) % 2

                def norm_a(hh, po):
                    # inline at block end: copy the (unnormalized) head
                    # output to bf16 and bounce the denominator row
                    # through DRAM into partition-major layout. Frees the
                    # po psum banks within ~2us, no slow ops in the path.
                    if hh == 0:
                        st["aot"] = aot_pool.tile(
                            [HD, 2, QT], BF16, name=f"aot{c}_{qi}"
                        )
                    nc.vector.tensor_copy(st["aot"][:, hh, :], po[0:HD, :])
                    nc.vector.tensor_copy(den_sb[0:1, hh, :], po[HD : HD + 1, :])
                    nc.sync.dma_start(dsc[sl, hh, :], den_sb[0:1, hh, :])
                    nc.sync.dma_start(
                        dent[:, sl, hh, :],
                        dsc[sl, hh, :].rearrange("(t p) -> p t", p=128),
                    )

                def mk_recip():
                    def go():
                        with nc.allow_low_precision(reason="denom recip"):
                            nc.vector.reciprocal(
                                dentr[:, sl, :, :], dent[:, sl, :, :]
                            )
                    return (0, go)

                def mk_fin(t4, hh):
                    def go():
                        ps = acc_pool.tile(
                            [128, D], F32, tag="acc", name=f"f{c}_{qi}_{t4}_{hh}"
                        )
                        nc.tensor.matmul(
                            ps,
                            st["aot"][:, hh, t4 * 128 : (t4 + 1) * 128],
                            wo_sb[:, 2 * c + hh, :],
                            start=True,
                            stop=True,
                        )
                        dst = out_acc[:, qi, t4, :]
                        den = dentr[:, sl, hh, t4 : t4 + 1]
                        other = bob_sb if (c == 0 and hh == 0) else dst
                        nc.vector.scalar_tensor_tensor(
                            dst, ps, den, other, op0=MUL, op1=ADD
                        )
                        if c == H // 2 - 1 and hh == 1:
                            t0 = qi * QT + t4 * 128
                            nc.sync.dma_start(out[t0 : t0 + 128, :], dst)
                    return (1, go)

                return norm_a, mk_recip, mk_fin

            # ---- pending-work queue: (mm_cost, closure), drained with a
            # per-iteration budget so PE load stays smooth.
            pending = []
            budget = [0.0]

            def drain(rate):
                budget[0] = min(budget[0] + rate, 4.0)
                while pending and pending[0][0] <= budget[0]:
                    cost, fn = pending.pop(0)
                    fn()
                    budget[0] -= cost

            # ---- prefix: minimum projections before attention (only
            # x^T segment 0 required, so PE starts as soon as the first
            # transposes land)
            for cl in q_pair_closures(0):
                cl[1]()
            for cl in k_pair_closures(0, 0):
                cl[1]()
            for j in range(4):
                v_unit(j)
            # remaining K chunk-0 units drain inside the first block
            pending.extend(
                (4, (lambda tt=tt: k_unit(0, tt))) for tt in range(2, 8)
            )

            # ---- attention ----
            for c in range(H // 2):
                for qi in range(NQT):
                    if qi == 1 and c + 1 < H // 2:
                        pending.extend(q_pair_closures(c + 1))
                        for p in range(4):
                            pending.extend(k_pair_closures(c + 1, p))
                    qs = qi * QT
                    po = [
                        po_pool.tile(
                            [HD + 1, QT], F32, tag="po", name=f"po{c}_{qi}_{hh}"
                        )
                        for hh in range(2)
                    ]

                    def scores_exp(k, qs=qs, c=c):
                        pss = sc_pool.tile([128, 2, QT], F32, tag="sc")
                        for hh in range(2):
                            off = hh * HD
                            nc.tensor.matmul(
                                pss[:, hh, :],
                                kt[off : off + HD, c, k * 128 : (k + 1) * 128],
                                qt[off : off + HD, c, qs : qs + QT],
                                start=True,
                                stop=True,
                            )
                        ptile = pt_pool.tile([128, 2, QT], BF16, tag="pt")
                        nc.scalar.activation(
                            ptile, pss, mybir.ActivationFunctionType.Exp,
                            scale=1.0 / np.sqrt(HD),
                        )
                        return ptile

                    first = c == 0 and qi == 0
                    ptile = scores_exp(0)
                    for k in range(KCH):
                        nxt = scores_exp(k + 1) if k + 1 < KCH else None
                        for hh in range(2):
                            nc.tensor.matmul(
                                po[hh],
                                v_sb[:, k, 2 * c + hh, :],
                                ptile[:, hh, :],
                                start=(k == 0),
                                stop=(k == KCH - 1),
                            )
                        ptile = nxt
                        if first and k < 28:
                            v_unit(k + 4)
                        drain(1.0 if k < 28 else 2.0)
                    # queue normalization + output projection for this block
                    norm_a, mk_recip, mk_f = normfin_closures(c, qi)
                    norm_a(0, po[0])
                    norm_a(1, po[1])
                    pending.append(mk_recip())
                    for t4 in range(QT // 128):
                        for hh in range(2):
                            pending.append(mk_f(t4, hh))
            while pending:
                pending.pop(0)[1]()

    nc.compile()
    return nc


_NC_CACHE = None


def _get_program():
    global _NC_CACHE
    if _NC_CACHE is None:
        _NC_CACHE = _build_program()
    return _NC_CACHE


def prepare_in_maps(x, Wq, bq, Wk, bk, Wv, bv, Wo, bo):
    bf = ml_dtypes.bfloat16
    x = np.ascontiguousarray(np.asarray(x, dtype=np.float32)).astype(bf)
    sh = {
        "wqt": np.ascontiguousarray(np.asarray(Wq, np.float32).T).astype(bf),
        "wkt": np.ascontiguousarray(np.asarray(Wk, np.float32).T).astype(bf),
        "wvt": np.ascontiguousarray(np.asarray(Wv, np.float32).T).astype(bf),
        "wos": np.ascontiguousarray(
            np.asarray(Wo, np.float32).T.reshape(H, HD, D).transpose(1, 0, 2)
        ).astype(bf),
        "bqs": np.ascontiguousarray(np.asarray(bq, np.float32).reshape(OC, 128).T),
        "bks": np.ascontiguousarray(np.asarray(bk, np.float32).reshape(OC, 128).T),
        "bvb": np.ascontiguousarray(
            np.broadcast_to(np.asarray(bv, np.float32), (128, D))
        ),
        "bob": np.ascontiguousarray(
            np.broadcast_to(np.asarray(bo, np.float32), (128, D))
        ),
    }
    in_maps = []
    for core in range(NCORES):
        b = core // (NCORES // B)
        qs = (core % (NCORES // B)) * QCH
        m = dict(sh)
        m["xq"] = np.ascontiguousarray(x[b, qs : qs + QCH, :])
        m["xkv"] = np.ascontiguousarray(x[b])
        in_maps.append(m)
    return in_maps


def assemble(results):
    out = np.empty((B, S, D), dtype=np.float32)
    for core in range(NCORES):
        b = core // (NCORES // B)
        qs = (core % (NCORES // B)) * QCH
        out[b, qs : qs + QCH, :] = results[core]["out"]
    return out


def kernel(x, Wq, bq, Wk, bk, Wv, bv, Wo, bo):
    in_maps = prepare_in_maps(x, Wq, bq, Wk, bk, Wv, bv, Wo, bo)
    nc = _get_program()
    res = run_bass_kernel_spmd(nc, in_maps, core_ids=list(range(NCORES)))
    return assemble(res.results)
